# revision 2
# baseline (speedup 1.0000x reference)
"""Fused multi-head attention (B=4, S=2048, D=1024, H=16, Dh=64, RoPE) on 8 NeuronCores.

Sharding: core = (batch b, head-group g) with b = core//2, g = core%2.
Each core computes its batch's 8 heads end-to-end; host sums the two
Wout row-slice partials per batch.

Key optimizations over the v1 kernel (448us -> ~396us):
  A. Flipped PV phase: out[q, dh] = et^T @ v_aug with queries on the output
     partitions. The moving operand is the fp16 v_aug [128, 65] (65 = Dh +
     ones column for the softmax denominator), so each matmul costs 65 rows
     instead of 512 -> the PV phase halves its PE time vs the v-stationary
     form. Cheap PE transposes ([64,128] x identity into fp16-bitcast PSUM
     slots) restore the [features, seq] layout the out-projection needs.
     HW constraint honored: a PSUM bank's accumulation context cannot be
     interleaved with another accumulation group in the same bank, so the
     8 slice-groups of an i_blk run contiguously per bank, trailing one
     i_blk behind their exp production (et tiles buffered in SBUF).
  B. RoPE rotate-half via four 32-row SBUF->SBUF DMAs per (pair, qk)
     (whole-row for pairs 1-3, per s-block for the latency-critical pair 0)
     instead of a DRAM fp16 round-trip: ~170 fewer DMA instructions on the
     serial HWDGE queue.
  C. Pair-interleaved wqk DRAM layout so each pair's q|k stationaries load
     with one DMA per k-tile; wout in fp16.
  D. Normalization via batched reciprocal (4 denominators per instr) +
     per-partition tensor_scalar multiply; all PSUM evacuations on DVE so
     ACT runs the exp stream nearly exclusively.
"""
import sys

for p in ("/opt/trn_rl_repo",):
    if p not in sys.path:
        sys.path.insert(0, p)

import contextlib
import numpy as np

import concourse.bacc as bacc
import concourse.bass as bass
import concourse.tile as tile
from concourse import mybir
from concourse.bass_utils import run_bass_kernel_spmd

P = 128
S = 2048
D = 1024
NH = 8            # heads per core
DH = 64
SB = 512          # matmul free-dim block
NSB = S // SB     # 4 s-blocks
KD = D // P       # 8 contraction tiles over d
ST = S // P       # 16 s partition-tiles (keys)
FV = NH * DH      # 512 features for this head group
N_CORES = 8
SCALE = DH ** -0.5

def _nullctx():
    return contextlib.nullcontext(None)


f32 = mybir.dt.float32
f32r = mybir.dt.float32r
bf16 = mybir.dt.float16  # fp16: 10-bit mantissa, values are O(10) so range is safe


def build_program(sim_bufs=2, aug_bufs=1, psa_bufs=2, depth=2, interleave=True):
    nc = bacc.Bacc("TRN2", target_bir_lowering=False, debug=False,
                   enable_asserts=False, num_devices=N_CORES)

    xT = nc.dram_tensor("xT", [D, S], f32r, kind="ExternalInput").ap()
    wqk = nc.dram_tensor("wqk", [D, 2 * FV], f32r, kind="ExternalInput").ap()
    wv = nc.dram_tensor("wv", [D, FV], f32r, kind="ExternalInput").ap()
    wout = nc.dram_tensor("wout", [FV, D], bf16, kind="ExternalInput").ap()
    cosb = nc.dram_tensor("cosb", [P, S], f32, kind="ExternalInput").ap()
    identt = nc.dram_tensor("identt", [P, P], bf16, kind="ExternalInput").ap()
    sinb = nc.dram_tensor("sinb", [P, S], f32, kind="ExternalInput").ap()
    outT = nc.dram_tensor("outT", [D, S], f32, kind="ExternalOutput").ap()

    with tile.TileContext(nc) as tc:
        with tc.tile_pool(name="persist", bufs=1) as pp, \
             tc.tile_pool(name="dram", bufs=1, space="DRAM") as dp, \
             tc.tile_pool(name="psum", bufs=1, space="PSUM") if interleave else _nullctx() as psp:
            v_sb = [pp.tile([P, NH * (DH + 1)], bf16, tag=f"v{i}", name=f"v{i}") for i in range(ST)]
            id_sb = pp.tile([P, P], bf16, tag="ident", name="id_sb")
            nc.sync.dma_start(id_sb[:], identt[:])
            tctr = [0]
            outT_sb = [[pp.tile([P, SB], bf16, tag=f"ot{t}_{ib}", name=f"ot{t}_{ib}")
                        for ib in range(NSB)] for t in range(NSB)]
            # rope products stay in SBUF; the rotate-half partition swap is
            # done with four SBUF->SBUF 32-row DMAs per (pair, qk, nb)

            PS = {"p": psp}

            def ps_tile(shape, tag, bufs, name):
                return PS["p"].tile(shape, f32, tag=tag, bufs=bufs, name=name)

            # ones columns of v_aug
            ones8 = pp.tile([P, NH], bf16, tag="ones8", name="ones8")
            nc.vector.memset(ones8[:], 1.0)
            for i in range(ST):
                ones_dst = v_sb[i].rearrange("p (h e) -> p h e", h=NH)[:, :, DH]
                nc.vector.tensor_copy(ones_dst, ones8[:])

            with tc.tile_pool(name="qkph", bufs=1) as qkph, \
                 tc.tile_pool(name="qks", bufs=1) as qks, \
                 tc.tile_pool(name="expp", bufs=18) as expp, \
                 tc.tile_pool(name="nump", bufs=2) as nump, \
                 tc.tile_pool(name="bcp", bufs=2) as bcp, \
                 tc.tile_pool(name="rrp", bufs=2) as rrp, \
                 tc.tile_pool(name="doutp", bufs=2) as doutp:

                wv_sb = [qkph.tile([P, FV], f32r, tag=f"wv{k}", name=f"wv{k}")
                         for k in range(KD)]

                def load_wv():
                    for k in range(KD):
                        nc.sync.dma_start(wv_sb[k][:], wv[P * k:P * (k + 1), :])

                def emit_a_setup(pairs):
                    # wqk dram is pair-interleaved: cols 256*t .. 256*t+256
                    # hold pair t's q|k stationaries -> one DMA per k-tile
                    wsl = {}
                    for pi, t in enumerate(pairs):
                        tiles = [qkph.tile([P, 2 * P], f32r, tag=f"w{pi}_{k}",
                                           name="wsl") for k in range(KD)]
                        for k in range(KD):
                            nc.sync.dma_start(
                                tiles[k][:],
                                wqk[P * k:P * (k + 1), 256 * t:256 * (t + 1)])
                        wsl[(t, 0)] = [tl[:, 0:P] for tl in tiles]
                        wsl[(t, 1)] = [tl[:, P:2 * P] for tl in tiles]
                    return wsl

                rope_t = {}

                def rope_tiles(t):
                    if t not in rope_t:
                        rope_t[t] = {qk: dict(
                            qc=qks.tile([P, S], bf16, tag=f"qc{qk}", bufs=2, name="qc"),
                            sn=qks.tile([P, S], bf16, tag=f"sn{qk}", bufs=2, name="sn"),
                            sw=qks.tile([P, S], bf16, tag=f"sw{qk}", bufs=2, name="sw"),
                            r=qks.tile([P, S], f32r, tag=f"r{qk}", bufs=2, name="r"),
                        ) for qk in range(2)}
                    return rope_t[t]

                def emit_a_nb(pairs, nb, wsl, with_v):
                    sl = slice(nb * SB, (nb + 1) * SB)
                    xts = [qkph.tile([P, SB], f32r, tag=f"xt{k}", bufs=1,
                                     name=f"xt{k}") for k in range(KD)]
                    for k in range(KD):
                        nc.sync.dma_start(xts[k][:], xT[P * k:P * (k + 1), sl])
                    cos_sb = qkph.tile([P, SB], f32, tag="cos", bufs=1, name="cos_sb")
                    sin_sb = qkph.tile([P, SB], f32, tag="sin", bufs=1, name="sin_sb")
                    nc.sync.dma_start(cos_sb[:], cosb[:, sl])
                    nc.sync.dma_start(sin_sb[:], sinb[:, sl])

                    for t in pairs:
                        rt = rope_tiles(t)
                        for qk in range(2):
                            ps = ps_tile([P, SB], "psA", psa_bufs, "ps")
                            for k in range(KD):
                                nc.tensor.matmul(ps[:], wsl[(t, qk)][k][:],
                                                 xts[k][:],
                                                 start=(k == 0), stop=(k == KD - 1))
                            d = rt[qk]
                            nc.vector.tensor_mul(d["sn"][:, sl], ps[:], sin_sb[:])
                            nc.vector.tensor_mul(d["qc"][:, sl], ps[:], cos_sb[:])
                            if 0 in pairs:
                                # pair0 is latency-critical: swap+add per nb
                                for blk in range(4):
                                    a = 32 * blk
                                    srow = 32 * (blk ^ 1)
                                    nc.sync.dma_start(d["sw"][a:a + 32, sl],
                                                      d["sn"][srow:srow + 32, sl])
                                nc.vector.tensor_add(d["r"][:, sl],
                                                     d["qc"][:, sl],
                                                     d["sw"][:, sl])

                    if with_v:
                        for st in range(NSB):
                            emit_av(xts, nb, st)
                    return xts

                def emit_av(xts, nb, st):
                    s_idx = nb * NSB + st
                    psv = ps_tile([P, FV], "psA", psa_bufs, "psv")
                    for k in range(KD):
                        nc.tensor.matmul(psv[:], xts[k][:, P * st:P * (st + 1)],
                                         wv_sb[k][:],
                                         start=(k == 0), stop=(k == KD - 1))
                    vdst = v_sb[s_idx].rearrange(
                        "p (h e) -> p h e", h=NH)[:, :, 0:DH]
                    vsrc = psv.rearrange("p (h e) -> p h e", h=NH)
                    nc.vector.tensor_copy(vdst, vsrc)

                def emit_a_group(pairs, with_v):
                    wsl = emit_a_setup(pairs)
                    for nb in range(NSB):
                        emit_a_nb(pairs, nb, wsl, with_v)
                    for t in pairs:
                        rt = rope_tiles(t)
                        for qk in range(2):
                            d = rt[qk]
                            for blk in range(4):
                                a = 32 * blk
                                srow = 32 * (blk ^ 1)
                                nc.sync.dma_start(d["sw"][a:a + 32, :],
                                                  d["sn"][srow:srow + 32, :])
                            nc.vector.tensor_add(d["r"][:], d["qc"][:],
                                                 d["sw"][:])

                def rope_pair_sb(t):
                    rt = rope_t[t]
                    return (rt[0]["r"], rt[1]["r"])

                def mk_cas():
                    return [ps_tile([P, 512], f"ca{ci}", 1, f"ca{ci}")
                            for ci in range(2)]

                def emit_norm(prev):
                    (tp, ip, cas, _ets) = prev
                    rcs = []
                    for ci in range(2):
                        rc = rrp.tile([P, 4], f32, tag="rrow", name="rc")
                        den = cas[ci][:, 0:260].rearrange(
                            "p (s e) -> p s e", e=65)[:, :, 64]
                        nc.vector.reciprocal(rc[:], den)
                        rcs.append(rc)
                    ca16 = [cc.bitcast(bf16) for cc in cas]
                    for c in range(4):
                        for hh in range(2):
                            si = 2 * (c % 2) + hh
                            ci = c // 2
                            nrm = nump.tile([P, DH], bf16, tag="num", name="nrm")
                            nc.vector.tensor_scalar_mul(
                                nrm[:], cas[ci][:, 65 * si:65 * si + 64],
                                rcs[ci][:, si:si + 1])
                            ts = tctr[0] % 4
                            tctr[0] += 1
                            tdst = ca16[ts // 2][0:DH,
                                                 520 + P * (ts % 2):520 + P * (ts % 2 + 1)]
                            nc.tensor.matmul(tdst, nrm[:], id_sb[:],
                                             is_transpose=True,
                                             skip_group_check=True)
                            dst = outT_sb[tp][ip][DH * hh:DH * (hh + 1),
                                                  P * c:P * (c + 1)]
                            nc.vector.tensor_copy(dst, tdst)

                def c_work(prev):
                    """C-matmul emission streams for the finished i_blk:
                    per PSUM bank the 4 slice-groups run back-to-back (a
                    bank's accumulation context cannot be interleaved with
                    another group in the same bank on hardware); the two
                    banks' streams interleave freely with everything else."""
                    (tp, ip, cas, ets) = prev

                    def acc_ap(hh, c):
                        si = 2 * (c % 2) + hh
                        return cas[c // 2][:, 65 * si:65 * si + 65]

                    def bank_stream(ci):
                        for c in (2 * ci, 2 * ci + 1):
                            for hh in range(2):
                                h = 2 * tp + hh
                                for j in range(ST):
                                    yield (acc_ap(hh, c),
                                           ets[j], SB * hh + P * c,
                                           v_sb[j][:, 65 * h:65 * h + 65],
                                           j)
                    work = []
                    for a, b in zip(bank_stream(0), bank_stream(1)):
                        work.append(a)
                        work.append(b)
                    return work

                def emit_cw(item):
                    ap, et, off, vsl, j = item
                    nc.tensor.matmul(ap, et[:, off:off + P], vsl,
                                     start=(j == 0), stop=(j == ST - 1),
                                     skip_group_check=True)

                def bcd_iblk(t, qs, ks, i_blk, prev, fill=None):
                    isl = slice(i_blk * SB, (i_blk + 1) * SB)
                    cas = mk_cas()
                    ets = {}
                    work = c_work(prev) if prev else []

                    def emit_b(j):
                        sim = ps_tile([P, 2 * SB], "sim", sim_bufs, "sim")
                        for hh in range(2):
                            off = DH * hh
                            nc.tensor.matmul(sim[:, SB * hh:SB * (hh + 1)],
                                             ks[off:off + DH, P * j:P * (j + 1)],
                                             qs[off:off + DH, isl],
                                             start=True, stop=True,
                                             tile_position=(DH * hh, 0))
                        et = expp.tile([P, 2 * SB], bf16, tag="exp", name="et")
                        nc.scalar.activation(et[:], sim[:],
                                             mybir.ActivationFunctionType.Exp,
                                             scale=SCALE)
                        ets[j] = et

                    for j in range(ST):
                        emit_b(j)
                        for _ in range(10):
                            if work:
                                emit_cw(work.pop(0))
                        if fill is not None and j % 2 == 1:
                            fill("j", t, i_blk)
                    while work:
                        emit_cw(work.pop(0))
                    if prev:
                        emit_norm(prev)
                    return (t, i_blk, cas, ets)

                def emit_bcd_pair(t, qs, ks, prev, fill=None):
                    for i_blk in range(NSB):
                        if fill is not None:
                            fill("iblk", t, i_blk)
                        prev = bcd_iblk(t, qs, ks, i_blk, prev, fill)
                    return prev

                def gen_pair0(state):
                    rt0 = rope_tiles(0)
                    r_q = rt0[0]["r"]
                    r_k = rt0[1]["r"]
                    cas = mk_cas()
                    ets = {}

                    def b0(j):
                        sim = ps_tile([P, 2 * SB], "sim", sim_bufs, "sim")
                        for hh in range(2):
                            off = DH * hh
                            nc.tensor.matmul(sim[:, SB * hh:SB * (hh + 1)],
                                             r_k[off:off + DH, P * j:P * (j + 1)],
                                             r_q[off:off + DH, 0:SB],
                                             start=True, stop=True,
                                             tile_position=(DH * hh, 0))
                        et = expp.tile([P, 2 * SB], bf16, tag="exp", name="et")
                        nc.scalar.activation(et[:], sim[:],
                                             mybir.ActivationFunctionType.Exp,
                                             scale=SCALE)
                        ets[j] = et

                    for nb in range(NSB):
                        yield
                        for j in range(4 * nb, 4 * nb + 4):
                            b0(j)
                    prev = (0, 0, cas, ets)
                    for i_blk in range(1, NSB):
                        prev = bcd_iblk(0, r_q, r_k, i_blk, prev)
                    state["prev"] = prev
                    state["rq"] = r_q
                    state["rk"] = r_k

                wout_sb = []

                def load_wout():
                    # wout reuses the wv slots (same shape, wv is dead after
                    # the v-sweep): tile (k, half) = wout[128k:+128, 512h:+512]
                    for k in range(FV // P):
                        for half in range(2):
                            w = qkph.tile([P, FV], bf16, tag=f"wv{2 * k + half}", name="wo")
                            nc.sync.dma_start(w[:],
                                              wout[P * k:P * (k + 1), FV * half:FV * (half + 1)])
                            wout_sb.append(w)

                def emit_d_group(mi, ib, tag, bufs, evac_eng):
                    isl = slice(ib * SB, (ib + 1) * SB)
                    pd = ps_tile([P, SB], tag, bufs, "pd")
                    for k in range(FV // P):
                        wt = wout_sb[2 * k + mi // 4]
                        nc.tensor.matmul(pd[:], wt[:, P * (mi % 4):P * (mi % 4 + 1)],
                                         outT_sb[k][ib][:],
                                         start=(k == 0), stop=(k == FV // P - 1))
                    ot = doutp.tile([P, SB], f32, tag="dout", name="dout")
                    if evac_eng == "dve":
                        nc.vector.tensor_copy(ot[:], pd[:])
                    else:
                        nc.scalar.copy(ot[:], pd[:])
                    nc.sync.dma_start(outT[P * mi:P * (mi + 1), isl], ot[:])

                _dq = []

                def d_filler(kind, t, i_blk):
                    # C (and hence outT) for i_blk K completes during K+1,
                    # so D groups trail two i_blks behind
                    if kind == "iblk" and i_blk >= 2:
                        _dq.extend((mi, i_blk - 2) for mi in range(D // P))
                    elif kind == "j" and _dq:
                        mi, ib = _dq.pop(0)
                        emit_d_group(mi, ib, "psA", psa_bufs, "dve")

                def emit_d_rest():
                    gi = 0
                    for mi, ib in _dq:
                        emit_d_group(mi, ib, "psA", psa_bufs,
                                     "dve")
                        gi += 1
                    _dq.clear()
                    for ib in (NSB - 2, NSB - 1):
                        for mi in range(D // P):
                            emit_d_group(mi, ib, "psA", psa_bufs,
                                         "dve")
                            gi += 1



                if interleave:
                    wsl0 = emit_a_setup((0,))
                    load_wv()
                    g0state = {}
                    g0 = gen_pair0(g0state)
                    next(g0)
                    for nb in range(NSB):
                        emit_a_nb((0,), nb, wsl0, with_v=True)
                        try:
                            next(g0)
                        except StopIteration:
                            pass
                    for _ in g0:
                        pass
                    prev = g0state["prev"]
                    emit_a_group((1,), with_v=False)
                    prev = emit_bcd_pair(1, *rope_pair_sb(1), prev)
                    emit_a_group((2, 3), with_v=False)
                    prev = emit_bcd_pair(2, *rope_pair_sb(2), prev)
                    r3 = rope_pair_sb(3)
                    load_wout()
                    prev = emit_bcd_pair(3, *r3, prev, fill=d_filler)
                    # drain: C of pair3-i3 interleaved with the D groups of
                    # i_blk 2 (whose outT is ready), then norm + final D
                    work = c_work(prev)
                    gi = 0
                    for mi in range(D // P):
                        for _ in range(12):
                            if work:
                                emit_cw(work.pop(0))
                        emit_d_group(mi, NSB - 2, "psA", psa_bufs,
                                     "dve")
                        gi += 1
                    while work:
                        emit_cw(work.pop(0))
                    emit_norm(prev)
                    for mi, ib in _dq:
                        emit_d_group(mi, ib, "psA", psa_bufs,
                                     "dve")
                        gi += 1
                    _dq.clear()
                    for mi in range(D // P):
                        emit_d_group(mi, NSB - 1, "psA", psa_bufs,
                                     "dve")
                        gi += 1
                else:
                    with tc.tile_pool(name="psA_ph", bufs=1, space="PSUM") as pa:
                        PS["p"] = pa
                        emit_a_group((0, 1), with_v=True)
                        emit_a_group((2, 3), with_v=False)
                    with tc.tile_pool(name="psB_ph", bufs=1, space="PSUM") as pb:
                        PS["p"] = pb
                        for t in range(NSB):
                            emit_bcd_pair(t, *rope_pair(t))
                    with tc.tile_pool(name="psD_ph", bufs=1, space="PSUM") as pdl:
                        PS["p"] = pdl
                        load_wout()
                        emit_d_rest()

    nc.compile()
    return nc


_PROG = None


def _get_prog():
    global _PROG
    if _PROG is None:
        _PROG = build_program()
    return _PROG


def make_in_maps(x, Wqkv, Wout):
    B = x.shape[0]
    HEADS = 16
    BASE = 10000.0
    # RoPE tables, sign folded into sin, 32-row frequency pattern tiled to 128
    f = np.arange(32, dtype=np.float64)
    invfreq = BASE ** (-2.0 * f / DH)                      # [32]
    tpos = np.arange(S, dtype=np.float64)
    ang = np.outer(invfreq, tpos)                          # [32, S]
    cos32 = np.cos(ang)
    sin32 = np.sin(ang)
    cosb = np.tile(cos32, (4, 1)).astype(np.float32)       # [128, S]
    # sign indexed by SOURCE row r: the swap moves row r to row swap(r), which
    # needs -sin when swap(r)%64 < 32, i.e. when r%64 >= 32
    sgn = np.repeat(np.array([1.0, -1.0, 1.0, -1.0]), 32)[:, None]
    sinb = (np.tile(sin32, (4, 1)) * sgn).astype(np.float32)
    identx = np.eye(128, dtype=np.float16)

    in_maps = []
    for c in range(N_CORES):
        b, g = divmod(c, 2)
        xTc = np.ascontiguousarray(x[b].T)                 # [D, S]
        cols = []
        for t in range(4):
            cols.append(Wqkv[:, 512 * g + 128 * t:512 * g + 128 * (t + 1)])
            cols.append(Wqkv[:, 1024 + 512 * g + 128 * t:1024 + 512 * g + 128 * (t + 1)])
        wqk_c = np.ascontiguousarray(np.concatenate(cols, axis=1))
        wv_c = np.ascontiguousarray(Wqkv[:, 2048 + 512 * g:2048 + 512 * g + 512])
        wout_c = np.ascontiguousarray(Wout[512 * g:512 * g + 512, :]).astype(np.float16)
        in_maps.append({"xT": xTc, "wqk": wqk_c, "wv": wv_c, "wout": wout_c,
                        "cosb": cosb, "sinb": sinb, "identt": identx})
    return in_maps


def gather_output(results, B=4):
    outs = []
    for b in range(B):
        acc = results[2 * b]["outT"].astype(np.float32) + results[2 * b + 1]["outT"]
        outs.append(acc.T)
    return np.stack(outs, axis=0)


def kernel(x, Wqkv, Wout):
    x = np.asarray(x, dtype=np.float32)
    Wqkv = np.asarray(Wqkv, dtype=np.float32)
    Wout = np.asarray(Wout, dtype=np.float32)
    nc = _get_prog()
    in_maps = make_in_maps(x, Wqkv, Wout)
    res = run_bass_kernel_spmd(nc, in_maps, core_ids=list(range(N_CORES)))
    return gather_output(res.results, B=x.shape[0])


if __name__ == "__main__":
    rng = np.random.default_rng(0)
    x = rng.standard_normal((4, S, D)).astype(np.float32)
    Wqkv = (rng.standard_normal((D, 3 * D)) * D ** -0.5).astype(np.float32)
    Wout = (rng.standard_normal((D, D)) * D ** -0.5).astype(np.float32)
    out = kernel(x, Wqkv, Wout)
    print("kernel ran, out shape:", out.shape, "finite:", np.isfinite(out).all())



# revision 3
# speedup vs baseline: 1.0141x; 1.0141x over previous
"""Fused multi-head attention (B=4, S=2048, D=1024, H=16, Dh=64, RoPE) on 8 NeuronCores.

Sharding: core = (batch b, head-group g) with b = core//2, g = core%2.
Each core computes its batch's 8 heads end-to-end; host sums the two
Wout row-slice partials per batch.

Key optimizations over the v1 kernel (448us -> ~396us):
  A. Flipped PV phase: out[q, dh] = et^T @ v_aug with queries on the output
     partitions. The moving operand is the fp16 v_aug [128, 65] (65 = Dh +
     ones column for the softmax denominator), so each matmul costs 65 rows
     instead of 512 -> the PV phase halves its PE time vs the v-stationary
     form. Cheap PE transposes ([64,128] x identity into fp16-bitcast PSUM
     slots) restore the [features, seq] layout the out-projection needs.
     HW constraint honored: a PSUM bank's accumulation context cannot be
     interleaved with another accumulation group in the same bank, so the
     8 slice-groups of an i_blk run contiguously per bank, trailing one
     i_blk behind their exp production (et tiles buffered in SBUF).
  B. RoPE rotate-half via four 32-row SBUF->SBUF DMAs per (pair, qk)
     (whole-row for pairs 1-3, per s-block for the latency-critical pair 0)
     instead of a DRAM fp16 round-trip: ~170 fewer DMA instructions on the
     serial HWDGE queue.
  C. Pair-interleaved wqk DRAM layout so each pair's q|k stationaries load
     with one DMA per k-tile; wout in fp16.
  D. Normalization via batched reciprocal (4 denominators per instr) +
     per-partition tensor_scalar multiply; all PSUM evacuations on DVE so
     ACT runs the exp stream nearly exclusively.
"""
import sys

for p in ("/opt/trn_rl_repo",):
    if p not in sys.path:
        sys.path.insert(0, p)

import contextlib
import numpy as np

import concourse.bacc as bacc
import concourse.bass as bass
import concourse.tile as tile
from concourse import mybir
from concourse.bass_utils import run_bass_kernel_spmd

P = 128
S = 2048
D = 1024
NH = 8            # heads per core
DH = 64
SB = 512          # matmul free-dim block
NSB = S // SB     # 4 s-blocks
KD = D // P       # 8 contraction tiles over d
ST = S // P       # 16 s partition-tiles (keys)
FV = NH * DH      # 512 features for this head group
N_CORES = 8
SCALE = DH ** -0.5

def _nullctx():
    return contextlib.nullcontext(None)


f32 = mybir.dt.float32
f32r = mybir.dt.float32r
bf16 = mybir.dt.float16  # fp16: 10-bit mantissa, values are O(10) so range is safe


def build_program(sim_bufs=2, aug_bufs=1, psa_bufs=2, depth=2, interleave=True):
    nc = bacc.Bacc("TRN2", target_bir_lowering=False, debug=False,
                   enable_asserts=False, num_devices=N_CORES)

    xT = nc.dram_tensor("xT", [D, S], f32r, kind="ExternalInput").ap()
    wqk = nc.dram_tensor("wqk", [D, 2 * FV], f32r, kind="ExternalInput").ap()
    wv = nc.dram_tensor("wv", [D, FV], f32r, kind="ExternalInput").ap()
    wout = nc.dram_tensor("wout", [FV, D], bf16, kind="ExternalInput").ap()
    cosb = nc.dram_tensor("cosb", [P, S], f32, kind="ExternalInput").ap()
    identt = nc.dram_tensor("identt", [P, P], bf16, kind="ExternalInput").ap()
    sinb = nc.dram_tensor("sinb", [P, S], f32, kind="ExternalInput").ap()
    outT = nc.dram_tensor("outT", [D, S], f32, kind="ExternalOutput").ap()

    with tile.TileContext(nc) as tc:
        with tc.tile_pool(name="persist", bufs=1) as pp, \
             tc.tile_pool(name="dram", bufs=1, space="DRAM") as dp, \
             tc.tile_pool(name="psum", bufs=1, space="PSUM") if interleave else _nullctx() as psp:
            v_sb = [pp.tile([P, NH * (DH + 1)], bf16, tag=f"v{i}", name=f"v{i}") for i in range(ST)]
            id_sb = pp.tile([P, P], bf16, tag="ident", name="id_sb")
            nc.sync.dma_start(id_sb[:], identt[:])
            tctr = [0]
            outT_sb = [[pp.tile([P, SB], bf16, tag=f"ot{t}_{ib}", name=f"ot{t}_{ib}")
                        for ib in range(NSB)] for t in range(NSB)]
            # rope products stay in SBUF; the rotate-half partition swap is
            # done with four SBUF->SBUF 32-row DMAs per (pair, qk, nb)

            PS = {"p": psp}

            def ps_tile(shape, tag, bufs, name):
                return PS["p"].tile(shape, f32, tag=tag, bufs=bufs, name=name)

            # ones columns of v_aug
            ones8 = pp.tile([P, NH], bf16, tag="ones8", name="ones8")
            nc.vector.memset(ones8[:], 1.0)
            for i in range(ST):
                ones_dst = v_sb[i].rearrange("p (h e) -> p h e", h=NH)[:, :, DH]
                nc.vector.tensor_copy(ones_dst, ones8[:])

            with tc.tile_pool(name="qkph", bufs=1) as qkph, \
                 tc.tile_pool(name="qks", bufs=1) as qks, \
                 tc.tile_pool(name="expp", bufs=18) as expp, \
                 tc.tile_pool(name="nump", bufs=2) as nump, \
                 tc.tile_pool(name="bcp", bufs=2) as bcp, \
                 tc.tile_pool(name="rrp", bufs=2) as rrp, \
                 tc.tile_pool(name="doutp", bufs=2) as doutp:

                wv_sb = [qkph.tile([P, FV], f32r, tag=f"wv{k}", name=f"wv{k}")
                         for k in range(KD)]

                wv_loaded = [False]

                def load_wv():
                    for k in range(KD):
                        nc.sync.dma_start(wv_sb[k][:], wv[P * k:P * (k + 1), :])

                def emit_a_setup(pairs):
                    # wqk dram is pair-interleaved: cols 256*t .. 256*t+256
                    # hold pair t's q|k stationaries -> one DMA per k-tile
                    wsl = {}
                    for pi, t in enumerate(pairs):
                        tiles = [qkph.tile([P, 2 * P], f32r, tag=f"w{pi}_{k}",
                                           name="wsl") for k in range(KD)]
                        for k in range(KD):
                            nc.sync.dma_start(
                                tiles[k][:],
                                wqk[P * k:P * (k + 1), 256 * t:256 * (t + 1)])
                        wsl[(t, 0)] = [tl[:, 0:P] for tl in tiles]
                        wsl[(t, 1)] = [tl[:, P:2 * P] for tl in tiles]
                    return wsl

                rope_t = {}

                def rope_tiles(t):
                    if t not in rope_t:
                        rope_t[t] = {qk: dict(
                            qc=qks.tile([P, S], bf16, tag=f"qc{qk}", bufs=2, name="qc"),
                            sn=qks.tile([P, S], bf16, tag=f"sn{qk}", bufs=2, name="sn"),
                            sw=qks.tile([P, S], bf16, tag=f"sw{qk}", bufs=2, name="sw"),
                            r=qks.tile([P, S], f32r, tag=f"r{qk}", bufs=2, name="r"),
                        ) for qk in range(2)}
                    return rope_t[t]

                def emit_a_nb(pairs, nb, wsl, with_v):
                    sl = slice(nb * SB, (nb + 1) * SB)
                    xts = [qkph.tile([P, SB], f32r, tag=f"xt{k}", bufs=1,
                                     name=f"xt{k}") for k in range(KD)]
                    for k in range(KD):
                        nc.sync.dma_start(xts[k][:], xT[P * k:P * (k + 1), sl])
                    cos_sb = qkph.tile([P, SB], f32, tag="cos", bufs=1, name="cos_sb")
                    sin_sb = qkph.tile([P, SB], f32, tag="sin", bufs=1, name="sin_sb")
                    nc.sync.dma_start(cos_sb[:], cosb[:, sl])
                    nc.sync.dma_start(sin_sb[:], sinb[:, sl])
                    if with_v and nb == 0 and not wv_loaded[0]:
                        # wv lands after pair0's gate data but before Av reads
                        load_wv()
                        wv_loaded[0] = True

                    for t in pairs:
                        rt = rope_tiles(t)
                        for qk in range(2):
                            ps = ps_tile([P, SB], "psA", psa_bufs, "ps")
                            for k in range(KD):
                                nc.tensor.matmul(ps[:], wsl[(t, qk)][k][:],
                                                 xts[k][:],
                                                 start=(k == 0), stop=(k == KD - 1))
                            d = rt[qk]
                            nc.vector.tensor_mul(d["sn"][:, sl], ps[:], sin_sb[:])
                            nc.vector.tensor_mul(d["qc"][:, sl], ps[:], cos_sb[:])
                            if 0 in pairs:
                                # pair0 is latency-critical: swap+add per nb
                                for blk in range(4):
                                    a = 32 * blk
                                    srow = 32 * (blk ^ 1)
                                    nc.sync.dma_start(d["sw"][a:a + 32, sl],
                                                      d["sn"][srow:srow + 32, sl])
                                nc.vector.tensor_add(d["r"][:, sl],
                                                     d["qc"][:, sl],
                                                     d["sw"][:, sl])

                    if with_v:
                        for st in range(NSB):
                            emit_av(xts, nb, st)
                    return xts

                def emit_av(xts, nb, st):
                    s_idx = nb * NSB + st
                    psv = ps_tile([P, FV], "psA", psa_bufs, "psv")
                    for k in range(KD):
                        nc.tensor.matmul(psv[:], xts[k][:, P * st:P * (st + 1)],
                                         wv_sb[k][:],
                                         start=(k == 0), stop=(k == KD - 1))
                    vdst = v_sb[s_idx].rearrange(
                        "p (h e) -> p h e", h=NH)[:, :, 0:DH]
                    vsrc = psv.rearrange("p (h e) -> p h e", h=NH)
                    nc.vector.tensor_copy(vdst, vsrc)

                def emit_a_group(pairs, with_v):
                    wsl = emit_a_setup(pairs)
                    for nb in range(NSB):
                        emit_a_nb(pairs, nb, wsl, with_v)
                    for t in pairs:
                        rt = rope_tiles(t)
                        for qk in range(2):
                            d = rt[qk]
                            for blk in range(4):
                                a = 32 * blk
                                srow = 32 * (blk ^ 1)
                                nc.sync.dma_start(d["sw"][a:a + 32, :],
                                                  d["sn"][srow:srow + 32, :])
                            nc.vector.tensor_add(d["r"][:], d["qc"][:],
                                                 d["sw"][:])

                def rope_pair_sb(t):
                    rt = rope_t[t]
                    return (rt[0]["r"], rt[1]["r"])

                def mk_cas():
                    return [ps_tile([P, 512], f"ca{ci}", 1, f"ca{ci}")
                            for ci in range(2)]

                def emit_norm(prev):
                    (tp, ip, cas, _ets) = prev
                    rcs = []
                    for ci in range(2):
                        rc = rrp.tile([P, 4], f32, tag="rrow", name="rc")
                        den = cas[ci][:, 0:260].rearrange(
                            "p (s e) -> p s e", e=65)[:, :, 64]
                        nc.vector.reciprocal(rc[:], den)
                        rcs.append(rc)
                    ca16 = [cc.bitcast(bf16) for cc in cas]
                    for c in range(4):
                        for hh in range(2):
                            si = 2 * (c % 2) + hh
                            ci = c // 2
                            nrm = nump.tile([P, DH], bf16, tag="num", name="nrm")
                            nc.vector.tensor_scalar_mul(
                                nrm[:], cas[ci][:, 65 * si:65 * si + 64],
                                rcs[ci][:, si:si + 1])
                            ts = tctr[0] % 4
                            tctr[0] += 1
                            tdst = ca16[ts // 2][0:DH,
                                                 520 + P * (ts % 2):520 + P * (ts % 2 + 1)]
                            nc.tensor.matmul(tdst, nrm[:], id_sb[:],
                                             is_transpose=True,
                                             skip_group_check=True)
                            dst = outT_sb[tp][ip][DH * hh:DH * (hh + 1),
                                                  P * c:P * (c + 1)]
                            nc.vector.tensor_copy(dst, tdst)

                def c_work(prev):
                    """C-matmul emission streams for the finished i_blk:
                    per PSUM bank the 4 slice-groups run back-to-back (a
                    bank's accumulation context cannot be interleaved with
                    another group in the same bank on hardware); the two
                    banks' streams interleave freely with everything else."""
                    (tp, ip, cas, ets) = prev

                    def acc_ap(hh, c):
                        si = 2 * (c % 2) + hh
                        return cas[c // 2][:, 65 * si:65 * si + 65]

                    def bank_stream(ci):
                        for c in (2 * ci, 2 * ci + 1):
                            for hh in range(2):
                                h = 2 * tp + hh
                                for j in range(ST):
                                    yield (acc_ap(hh, c),
                                           ets[j], SB * hh + P * c,
                                           v_sb[j][:, 65 * h:65 * h + 65],
                                           j)
                    work = []
                    for a, b in zip(bank_stream(0), bank_stream(1)):
                        work.append(a)
                        work.append(b)
                    return work

                def emit_cw(item):
                    ap, et, off, vsl, j = item
                    nc.tensor.matmul(ap, et[:, off:off + P], vsl,
                                     start=(j == 0), stop=(j == ST - 1),
                                     skip_group_check=True)

                def bcd_iblk(t, qs, ks, i_blk, prev, fill=None):
                    isl = slice(i_blk * SB, (i_blk + 1) * SB)
                    cas = mk_cas()
                    ets = {}
                    work = c_work(prev) if prev else []

                    def emit_b(j):
                        sim = ps_tile([P, 2 * SB], "sim", sim_bufs, "sim")
                        for hh in range(2):
                            off = DH * hh
                            nc.tensor.matmul(sim[:, SB * hh:SB * (hh + 1)],
                                             ks[off:off + DH, P * j:P * (j + 1)],
                                             qs[off:off + DH, isl],
                                             start=True, stop=True,
                                             tile_position=(DH * hh, 0))
                        et = expp.tile([P, 2 * SB], bf16, tag="exp", name="et")
                        nc.scalar.activation(et[:], sim[:],
                                             mybir.ActivationFunctionType.Exp,
                                             scale=SCALE)
                        ets[j] = et

                    for j in range(ST):
                        emit_b(j)
                        for _ in range(10):
                            if work:
                                emit_cw(work.pop(0))
                        if fill is not None and j % 2 == 1:
                            fill("j", t, i_blk)
                    while work:
                        emit_cw(work.pop(0))
                    if prev:
                        emit_norm(prev)
                    return (t, i_blk, cas, ets)

                def emit_bcd_pair(t, qs, ks, prev, fill=None):
                    for i_blk in range(NSB):
                        if fill is not None:
                            fill("iblk", t, i_blk)
                        prev = bcd_iblk(t, qs, ks, i_blk, prev, fill)
                    return prev

                def gen_pair0(state):
                    rt0 = rope_tiles(0)
                    r_q = rt0[0]["r"]
                    r_k = rt0[1]["r"]
                    cas = mk_cas()
                    ets = {}

                    def b0(j):
                        sim = ps_tile([P, 2 * SB], "sim", sim_bufs, "sim")
                        for hh in range(2):
                            off = DH * hh
                            nc.tensor.matmul(sim[:, SB * hh:SB * (hh + 1)],
                                             r_k[off:off + DH, P * j:P * (j + 1)],
                                             r_q[off:off + DH, 0:SB],
                                             start=True, stop=True,
                                             tile_position=(DH * hh, 0))
                        et = expp.tile([P, 2 * SB], bf16, tag="exp", name="et")
                        nc.scalar.activation(et[:], sim[:],
                                             mybir.ActivationFunctionType.Exp,
                                             scale=SCALE)
                        ets[j] = et

                    for nb in range(NSB):
                        yield
                        for j in range(4 * nb, 4 * nb + 4):
                            b0(j)
                    prev = (0, 0, cas, ets)
                    for i_blk in range(1, NSB):
                        prev = bcd_iblk(0, r_q, r_k, i_blk, prev)
                    state["prev"] = prev
                    state["rq"] = r_q
                    state["rk"] = r_k

                wout_sb = []

                def load_wout():
                    # wout reuses the wv slots (same shape, wv is dead after
                    # the v-sweep): tile (k, half) = wout[128k:+128, 512h:+512]
                    for k in range(FV // P):
                        for half in range(2):
                            w = qkph.tile([P, FV], bf16, tag=f"wv{2 * k + half}", name="wo")
                            nc.sync.dma_start(w[:],
                                              wout[P * k:P * (k + 1), FV * half:FV * (half + 1)])
                            wout_sb.append(w)

                def emit_d_group(mi, ib, tag, bufs, evac_eng):
                    isl = slice(ib * SB, (ib + 1) * SB)
                    pd = ps_tile([P, SB], tag, bufs, "pd")
                    for k in range(FV // P):
                        wt = wout_sb[2 * k + mi // 4]
                        nc.tensor.matmul(pd[:], wt[:, P * (mi % 4):P * (mi % 4 + 1)],
                                         outT_sb[k][ib][:],
                                         start=(k == 0), stop=(k == FV // P - 1))
                    ot = doutp.tile([P, SB], f32, tag="dout", name="dout")
                    if evac_eng == "dve":
                        nc.vector.tensor_copy(ot[:], pd[:])
                    else:
                        nc.scalar.copy(ot[:], pd[:])
                    nc.sync.dma_start(outT[P * mi:P * (mi + 1), isl], ot[:])

                _dq = []

                def d_filler(kind, t, i_blk):
                    # C (and hence outT) for i_blk K completes during K+1,
                    # so D groups trail two i_blks behind
                    if kind == "iblk" and i_blk >= 2:
                        _dq.extend((mi, i_blk - 2) for mi in range(D // P))
                    elif kind == "j" and _dq:
                        mi, ib = _dq.pop(0)
                        emit_d_group(mi, ib, "psA", psa_bufs, "dve")

                def emit_d_rest():
                    gi = 0
                    for mi, ib in _dq:
                        emit_d_group(mi, ib, "psA", psa_bufs,
                                     "dve")
                        gi += 1
                    _dq.clear()
                    for ib in (NSB - 2, NSB - 1):
                        for mi in range(D // P):
                            emit_d_group(mi, ib, "psA", psa_bufs,
                                         "dve")
                            gi += 1



                if interleave:
                    warm = ps_tile([P, 2 * SB], "sim", sim_bufs, "warm")
                    for _ in range(60):
                        nc.tensor.matmul(warm[:, 0:P], id_sb[:], id_sb[:],
                                         start=True, stop=True,
                                         skip_group_check=True)
                    wsl0 = emit_a_setup((0,))
                    g0state = {}
                    g0 = gen_pair0(g0state)
                    next(g0)
                    for nb in range(NSB):
                        emit_a_nb((0,), nb, wsl0, with_v=True)
                        try:
                            next(g0)
                        except StopIteration:
                            pass
                    for _ in g0:
                        pass
                    prev = g0state["prev"]
                    emit_a_group((1,), with_v=False)
                    prev = emit_bcd_pair(1, *rope_pair_sb(1), prev)
                    emit_a_group((2, 3), with_v=False)
                    prev = emit_bcd_pair(2, *rope_pair_sb(2), prev)
                    r3 = rope_pair_sb(3)
                    load_wout()
                    prev = emit_bcd_pair(3, *r3, prev, fill=d_filler)
                    # drain: C of pair3-i3 interleaved with the D groups of
                    # i_blk 2 (whose outT is ready), then norm + final D
                    work = c_work(prev)
                    gi = 0
                    for mi in range(D // P):
                        for _ in range(12):
                            if work:
                                emit_cw(work.pop(0))
                        emit_d_group(mi, NSB - 2, "psA", psa_bufs,
                                     "dve")
                        gi += 1
                    while work:
                        emit_cw(work.pop(0))
                    emit_norm(prev)
                    for mi, ib in _dq:
                        emit_d_group(mi, ib, "psA", psa_bufs,
                                     "dve")
                        gi += 1
                    _dq.clear()
                    for mi in range(D // P):
                        emit_d_group(mi, NSB - 1, "psA", psa_bufs,
                                     "dve")
                        gi += 1
                else:
                    with tc.tile_pool(name="psA_ph", bufs=1, space="PSUM") as pa:
                        PS["p"] = pa
                        emit_a_group((0, 1), with_v=True)
                        emit_a_group((2, 3), with_v=False)
                    with tc.tile_pool(name="psB_ph", bufs=1, space="PSUM") as pb:
                        PS["p"] = pb
                        for t in range(NSB):
                            emit_bcd_pair(t, *rope_pair(t))
                    with tc.tile_pool(name="psD_ph", bufs=1, space="PSUM") as pdl:
                        PS["p"] = pdl
                        load_wout()
                        emit_d_rest()

    nc.compile()
    return nc


_PROG = None


def _get_prog():
    global _PROG
    if _PROG is None:
        _PROG = build_program()
    return _PROG


def make_in_maps(x, Wqkv, Wout):
    B = x.shape[0]
    HEADS = 16
    BASE = 10000.0
    # RoPE tables, sign folded into sin, 32-row frequency pattern tiled to 128
    f = np.arange(32, dtype=np.float64)
    invfreq = BASE ** (-2.0 * f / DH)                      # [32]
    tpos = np.arange(S, dtype=np.float64)
    ang = np.outer(invfreq, tpos)                          # [32, S]
    cos32 = np.cos(ang)
    sin32 = np.sin(ang)
    cosb = np.tile(cos32, (4, 1)).astype(np.float32)       # [128, S]
    # sign indexed by SOURCE row r: the swap moves row r to row swap(r), which
    # needs -sin when swap(r)%64 < 32, i.e. when r%64 >= 32
    sgn = np.repeat(np.array([1.0, -1.0, 1.0, -1.0]), 32)[:, None]
    sinb = (np.tile(sin32, (4, 1)) * sgn).astype(np.float32)
    identx = np.eye(128, dtype=np.float16)

    in_maps = []
    for c in range(N_CORES):
        b, g = divmod(c, 2)
        xTc = np.ascontiguousarray(x[b].T)                 # [D, S]
        cols = []
        for t in range(4):
            cols.append(Wqkv[:, 512 * g + 128 * t:512 * g + 128 * (t + 1)])
            cols.append(Wqkv[:, 1024 + 512 * g + 128 * t:1024 + 512 * g + 128 * (t + 1)])
        wqk_c = np.ascontiguousarray(np.concatenate(cols, axis=1))
        wv_c = np.ascontiguousarray(Wqkv[:, 2048 + 512 * g:2048 + 512 * g + 512])
        wout_c = np.ascontiguousarray(Wout[512 * g:512 * g + 512, :]).astype(np.float16)
        in_maps.append({"xT": xTc, "wqk": wqk_c, "wv": wv_c, "wout": wout_c,
                        "cosb": cosb, "sinb": sinb, "identt": identx})
    return in_maps


def gather_output(results, B=4):
    outs = []
    for b in range(B):
        acc = results[2 * b]["outT"].astype(np.float32) + results[2 * b + 1]["outT"]
        outs.append(acc.T)
    return np.stack(outs, axis=0)


def kernel(x, Wqkv, Wout):
    x = np.asarray(x, dtype=np.float32)
    Wqkv = np.asarray(Wqkv, dtype=np.float32)
    Wout = np.asarray(Wout, dtype=np.float32)
    nc = _get_prog()
    in_maps = make_in_maps(x, Wqkv, Wout)
    res = run_bass_kernel_spmd(nc, in_maps, core_ids=list(range(N_CORES)))
    return gather_output(res.results, B=x.shape[0])


if __name__ == "__main__":
    rng = np.random.default_rng(0)
    x = rng.standard_normal((4, S, D)).astype(np.float32)
    Wqkv = (rng.standard_normal((D, 3 * D)) * D ** -0.5).astype(np.float32)
    Wout = (rng.standard_normal((D, D)) * D ** -0.5).astype(np.float32)
    out = kernel(x, Wqkv, Wout)
    print("kernel ran, out shape:", out.shape, "finite:", np.isfinite(out).all())

